# revision 1
# baseline (speedup 1.0000x reference)
"""Trainium2 Bass kernel for ChannelAttention (B=16, C=512, H=W=64).

Math (per batch b):
    xf = x[b] reshaped [C, N], N = H*W = 4096
    q = Wq @ xf + bq            [64, N]
    k = Wk @ xf + bk            [64, N]
    v = Wv @ xf + bv            [64, N]
    energy = q @ k.T            [64, 64]   (contraction over N)
    attn = softmax(energy, -1)
    z = attn @ v                [64, N]
    out = Wo @ z + bo           [C, N]

Sharding: data-parallel over batch, 2 batches per core on 8 cores, no
collectives.  Each core receives its x shard plus the (host-pre-transposed)
weights and returns its out shard.

On-chip dataflow per batch (8 n-panels of 512), default scheme "b":
  - qT|kT projected DIRECTLY in transposed [n, q|k] layout: per 128-wide
    n-subtile, 4 accumulating matmuls with the xf c-chunk as the stationary
    operand (lhsT) and [WqT|WkT] as the moving operand.  This avoids any
    explicit transposes; biases are added along the free dim with a
    broadcast tile during the PSUM->SBUF copy on DVE.  The energy
    [64, 64] accumulates over all 32 n-subtiles as qT.T @ kT in full fp32
    (softmax is sensitive to energy error: values are ~N(0, 64^2), so the
    top-2 gap can be small and tf32-level error would be amplified).
  - v projected in native [64, n] layout (float32r, 1 cycle/row), kept in
    SBUF for the whole batch.
  - softmax: DVE row-max (negated), ACT exp with bias=-max and accum_out
    row-sum, DVE reciprocal + row scale of attn in place.
  - out = Wo @ (attn @ v) + bo is reassociated as (Wo @ attn) @ v:
    W2T = attn.T-free matmul (lhsT=attn native, rhs=WoT, one instruction),
    then out m-tiles = W2T-slice.T @ v panels in float32r, bias added on
    DVE during the PSUM->SBUF copy, DMA'd out per [128, 512] tile.

Matmul dtype notes: float32 is exact but runs at 4 cycles/row on the PE;
float32r runs at 1 cycle/row (for free dim >= 256) with ~tf32 multiply
precision (measured ~5e-4 relative on this problem's linear paths).
fp32r operands must be *typed* float32r at their producer (DMA from an
fp32r DRAM tensor, or an ACT/DVE copy with fp32r output); the bytes are
plain fp32 and can be bitcast back for exact fp32 consumers.
Measured on HW: rel L2 error 6.2e-4 vs the fp32 reference; ~150-180 us
per-core device time (2 batches/core), vs a ~70 us pure-DMA floor.
"""

import os

import numpy as np

# Problem shape (hardcoded; kernel.py must be self-contained).
B, C, H, W = 16, 512, 64, 64
N = H * W  # 4096
C8 = 64
P = 128
NCORES = 8
BPC = B // NCORES  # batches per core
CCH = C // P  # 4 c-chunks of 128
NP = 512  # n-panel width
NPANELS = N // NP  # 8
NSUB = NP // P  # 4 transpose subtiles per panel

# Matmul dtype knobs: "f32" = exact 4 cyc/row; "f32r" = tf32-ish, 1 cyc/row
# only for free dim >= 256; "f16" = same 11-bit mantissa as tf32 but
# 1 cyc/row at ANY free dim, and half the DMA/SBUF bytes.
QK_DT = os.environ.get("CHATT_QK_DT", "f16")
V_DT = os.environ.get("CHATT_V_DT", "f16")
EN_DT = os.environ.get("CHATT_EN_DT", "f16")
ZO_DT = os.environ.get("CHATT_ZO_DT", "f16")
# Output DRAM dtype: f16/bf16 halve the store traffic (host converts back
# to f32); fp16 rel-L2 impact ~3e-4, far under the 2e-2 gate.
Y_DT = os.environ.get("CHATT_Y_DT", "f16")
# Timing aid: repeat the whole body REPS times inside a hardware loop so the
# device time is measurable above the host<->device transfer noise.
REPS = int(os.environ.get("CHATT_REPS", "1"))
# Bisection aids (timing experiments only; outputs become wrong):
SKIP_ENERGY = os.environ.get("CHATT_SKIP_ENERGY", "0") == "1"
SKIP_PHASEB = os.environ.get("CHATT_SKIP_PHASEB", "0") == "1"
# Energy-path structure:
#  "t": project q|k in native layout, PE-transpose panels, energy from qkT
#      (with QK_DT=f32r the projection runs 1 cycle/row: free dim 512)
#  "b": project qT|kT directly (xf chunks as stationary operand) - fewer
#       cross-engine hops, no transposes; needs QK_DT=f16 to hit
#       1 cycle/row (fp32/fp32r both run 4 cycles/row at free dim 128)
SCHEME = os.environ.get("CHATT_SCHEME", "b")
# Engine for the out-tile PSUM->SBUF bias copies: "dve", "act", "alt"
# (dve/act alternating), or "pd" (dve/pool alternating; pool = gpsimd)
OUT_ENG = os.environ.get("CHATT_OUT_ENG", "dve")
# Engine for the scheme-t qkT PSUM->SBUF copies: "dve" or "pool"
QKT_ENG = os.environ.get("CHATT_QKT_ENG", "dve")
# Out-DMA granularity: "mtile" = [128,512] per (mo,panel); "panel" = staged
# [512,512] per panel; "batch" = one [512, 4096] DMA per batch (8KB rows)
OUT_STAGE = os.environ.get("CHATT_OUT_STAGE", "mtile")
# Input DMA granularity: panels per dma_start (1 -> 1MB, 2 -> 2MB)
XF_PANELS = int(os.environ.get("CHATT_XF_PANELS", "2"))
# Fold the out bias into the out matmul by augmenting the contraction dim
# (K=64 -> 65: ones row in v_sb, bo row in w2_sb).  Free on the PE (matmul
# cost is keyed on the free dim) and turns the out-tile PSUM->SBUF bias
# adds into plain copies.
BFOLD = os.environ.get("CHATT_BFOLD", "0") == "1"

_CACHE = {}
LAST_RESULTS = None


def _build_program():
    import concourse.bass as bass  # noqa: F401
    import concourse.mybir as mybir
    import concourse.tile as tile
    from concourse import bacc
    from concourse.masks import make_identity
    from contextlib import ExitStack

    f32 = mybir.dt.float32
    f32r = mybir.dt.float32r
    bf16 = mybir.dt.bfloat16
    f16 = mybir.dt.float16
    y_dt = {"bf16": bf16, "f16": f16}.get(Y_DT, f32)

    def dt_of(kind):
        return {"bf16": bf16, "f16": f16, "f32r": f32r}.get(kind, f32)

    # xf feeds both the qk and v projections.  16-bit x (half the load
    # traffic) requires both consumers 16-bit; otherwise it is typed f32r
    # if either consumer is f32r and bitcast back to f32 for an exact
    # consumer (fp32r bytes are fp32 bytes).
    dsz = mybir.dt.size
    if dsz(dt_of(QK_DT)) == 2 or dsz(dt_of(V_DT)) == 2:
        assert dt_of(QK_DT) == dt_of(V_DT), (
            "16-bit x must feed both the qk and v projections"
        )
        xf_dt = dt_of(QK_DT)
    else:
        xf_dt = f32r if (QK_DT == "f32r" or V_DT == "f32r") else f32

    def x_cast(ap, kind):
        # cast xf slice to the dtype wanted by this matmul
        want = dt_of(kind)
        if ap.dtype == want:
            return ap
        assert dsz(ap.dtype) == dsz(want), (QK_DT, V_DT)
        return ap.bitcast(want)

    nc = bacc.Bacc("TRN2", target_bir_lowering=False)

    x_h = nc.dram_tensor("x", [BPC, C, N], xf_dt, kind="ExternalInput")
    wqk_h = nc.dram_tensor("w_qkt", [C, P], dt_of(QK_DT), kind="ExternalInput")
    wv_h = nc.dram_tensor("w_vt", [C, C8], dt_of(V_DT), kind="ExternalInput")
    wo_h = nc.dram_tensor("w_ot", [C8, C], dt_of(ZO_DT), kind="ExternalInput")
    bqk_h = nc.dram_tensor("b_qk", [P], f32, kind="ExternalInput")
    bv_h = nc.dram_tensor("b_v", [C8], f32, kind="ExternalInput")
    bo_h = nc.dram_tensor("b_o", [C], f32, kind="ExternalInput")
    y_h = nc.dram_tensor("y", [BPC, C, N], y_dt, kind="ExternalOutput")

    x_ap = x_h.ap()
    y_ap = y_h.ap()

    with tile.TileContext(nc) as tc, ExitStack() as ctx:
        def _n(name, default):
            return int(os.environ.get(f"CHATT_BUFS_{name}", str(default)))

        consts = ctx.enter_context(tc.tile_pool(name="consts", bufs=1))
        xp = ctx.enter_context(
            tc.tile_pool(name="xp", bufs=_n("XP", max(2, 8 // XF_PANELS)))
        )
        qkp = ctx.enter_context(tc.tile_pool(name="qkp", bufs=_n("QKP", 3)))
        qktp = ctx.enter_context(tc.tile_pool(name="qktp", bufs=_n("QKTP", 4)))
        vp = ctx.enter_context(tc.tile_pool(name="vp", bufs=2))
        zp = ctx.enter_context(tc.tile_pool(name="zp", bufs=3))
        op = ctx.enter_context(
            tc.tile_pool(
                name="op",
                bufs=_n(
                    "OP",
                    {"mtile": 6, "panel": 3, "batch": 2}.get(OUT_STAGE, 6),
                ),
            )
        )
        smallp = ctx.enter_context(tc.tile_pool(name="smallp", bufs=4))
        # PSUM: 8 banks total.
        # scheme t: proj(qk+v) 3 + transpose 2 + energy 1 + out 2
        # scheme b: proj(v) 2 + qkT 3 + energy 1 + out 2
        ps_cfg = os.environ.get("CHATT_PSUM", "e2")
        pe_n = 1
        if ps_cfg == "b":
            pp_n, pt_n, pzo_n = (2, 3, 2)
        elif ps_cfg == "e2":
            # double-buffer the energy bank so batch b+1's energy
            # accumulation doesn't wait for batch b's softmax
            pp_n, pt_n, pe_n, pzo_n = 2, 2, 2, 2
        elif ps_cfg == "o3":
            # triple-buffer the out banks for more out-matmul ILP
            pp_n, pt_n, pe_n, pzo_n = 1, 2, 2, 3
        else:
            pp_n, pt_n, pzo_n = (3, 2, 2)
        pp = ctx.enter_context(tc.tile_pool(name="pp", bufs=pp_n, space="PSUM"))
        pt = ctx.enter_context(tc.tile_pool(name="pt", bufs=pt_n, space="PSUM"))
        pe = ctx.enter_context(tc.tile_pool(name="pe", bufs=pe_n, space="PSUM"))
        pzo = ctx.enter_context(tc.tile_pool(name="pzo", bufs=pzo_n, space="PSUM"))

        # One-time constants.
        wqk_sb = consts.tile([P, CCH, P], dt_of(QK_DT))
        nc.sync.dma_start(wqk_sb, wqk_h.ap().rearrange("(co ci) m -> ci co m", ci=P))
        wv_sb = consts.tile([P, CCH, C8], dt_of(V_DT))
        nc.sync.dma_start(wv_sb, wv_h.ap().rearrange("(co ci) m -> ci co m", ci=P))
        wo_sb = consts.tile([C8, C], dt_of(ZO_DT))
        nc.sync.dma_start(wo_sb, wo_h.ap())
        bqk_sb = consts.tile([P, 1], f32)
        nc.sync.dma_start(bqk_sb, bqk_h.ap()[:, None])
        bv_sb = consts.tile([C8, 1], f32)
        nc.sync.dma_start(bv_sb, bv_h.ap()[:, None])
        bo_sb = consts.tile([P, CCH], f32)
        nc.sync.dma_start(bo_sb, bo_h.ap().rearrange("(mo mi) -> mi mo", mi=P))
        if BFOLD:
            # bo as a [1, C] row in the out-matmul dtype, for the K=65 fold
            bo_row32 = consts.tile([1, C], f32)
            nc.sync.dma_start(bo_row32, bo_h.ap()[None, :])
            bo_row = consts.tile([1, C], dt_of(ZO_DT))
            nc.vector.tensor_copy(bo_row, bo_row32)
        ident = consts.tile([P, P], f32)
        make_identity(nc, ident)
        ident_r = None
        if SCHEME == "t" and dt_of(QK_DT) != f32:
            # fp32r operands must be *typed* fp32r at their producer; a
            # bitcast of the f32 identity is rejected by the BIR verifier,
            # and memset can't write f32r, so copy through DVE.
            ident_r = consts.tile([P, P], dt_of(QK_DT))
            nc.vector.tensor_copy(ident_r, ident)
        if SCHEME == "b":
            # b_qk broadcast to all partitions: [128, 128] with the bias
            # along the free dim (for the transposed-layout bias add)
            bqk_bc = consts.tile([P, P], f32)
            nc.sync.dma_start(
                bqk_bc,
                bass.AP(tensor=bqk_h, offset=0, ap=[[0, P], [1, P]]),
            )

        Identity = mybir.ActivationFunctionType.Identity
        Copy = mybir.ActivationFunctionType.Copy
        Exp = mybir.ActivationFunctionType.Exp

        from contextlib import nullcontext

        hint = (
            (
                mybir.EngineType.PE,
                mybir.EngineType.Activation,
                mybir.EngineType.DVE,
                mybir.EngineType.SP,
            )
            if os.environ.get("CHATT_HINT", "0") == "1"
            else ()
        )
        rep_cm = (
            tc.For_i(0, REPS, 1, hint_engines=hint) if REPS > 1 else nullcontext()
        )
        with rep_cm:
            for b in range(BPC):
                xb = x_ap[b].rearrange("(co ci) n -> ci co n", ci=P)
                yb = y_ap[b].rearrange("(mo mi) n -> mi mo n", mi=P)

                energy = pe.tile([C8, C8], f32, tag="energy", name=f"energy_{b}")
                KV = C8 + 1 if BFOLD else C8
                v_full = vp.tile([KV, N], dt_of(ZO_DT), tag="v", name=f"v_{b}")
                v_sb = v_full[0:C8, :]
                if BFOLD:
                    # ones row for the K=65 out-bias fold
                    nc.gpsimd.memset(v_full[C8 : C8 + 1, :], 1.0)

                # ---- Phase A: projections + energy accumulation ----
                xf_group = {}
                for p in range(NPANELS):
                    nsl = slice(p * NP, (p + 1) * NP)
                    if p % XF_PANELS == 0:
                        gw = XF_PANELS * NP
                        xf_g = xp.tile(
                            [P, CCH, gw], xf_dt, tag="xf", name=f"xf_{b}_{p}"
                        )
                        nc.sync.dma_start(
                            xf_g, xb[:, :, p * NP : p * NP + gw]
                        )
                        xf_group = {"tile": xf_g, "base": p}
                    off = (p - xf_group["base"]) * NP
                    xf = xf_group["tile"][:, :, off : off + NP]

                    v_ps = pp.tile([C8, NP], f32, tag="proj", name=f"vps_{b}_{p}")
                    for co in range(CCH):
                        nc.tensor.matmul(
                            v_ps,
                            wv_sb[:, co, :],
                            x_cast(xf[:, co, :], V_DT),
                            start=(co == 0),
                            stop=(co == CCH - 1),
                        )
                    nc.scalar.activation(
                        v_sb[:, nsl], v_ps, Identity, bias=bv_sb, scale=1.0
                    )

                    last_p = 0 if SKIP_ENERGY else NPANELS - 1
                    if SCHEME == "b":
                        if not (SKIP_ENERGY and p > 0):
                            for ns in range(NSUB):
                                qt_ps = pt.tile(
                                    [P, P], f32, tag="tp", name=f"qtps_{b}_{p}_{ns}"
                                )
                                for co in range(CCH):
                                    nc.tensor.matmul(
                                        qt_ps,
                                        x_cast(
                                            xf[:, co, ns * P : (ns + 1) * P], QK_DT
                                        ),
                                        wqk_sb[:, co, :],
                                        start=(co == 0),
                                        stop=(co == CCH - 1),
                                    )
                                qkt_sb = qktp.tile(
                                    [P, P],
                                    dt_of(EN_DT),
                                    tag="qkt",
                                    name=f"qkt_{b}_{p}_{ns}",
                                )
                                qkt_eng = (
                                    nc.gpsimd if QKT_ENG == "pool" else nc.vector
                                )
                                qkt_eng.tensor_tensor(
                                    qkt_sb, qt_ps, bqk_bc, mybir.AluOpType.add
                                )
                                nc.tensor.matmul(
                                    energy,
                                    qkt_sb[:, 0:C8],
                                    qkt_sb[:, C8:P],
                                    start=(p == 0 and ns == 0),
                                    stop=(p == last_p and ns == NSUB - 1),
                                )
                    else:
                        qk_ps = pp.tile([P, NP], f32, tag="proj", name=f"qkps_{b}_{p}")
                        for co in range(CCH):
                            nc.tensor.matmul(
                                qk_ps,
                                wqk_sb[:, co, :],
                                x_cast(xf[:, co, :], QK_DT),
                                start=(co == 0),
                                stop=(co == CCH - 1),
                            )
                        qk_sb = qkp.tile(
                            [P, NP], dt_of(QK_DT), tag="qk", name=f"qk_{b}_{p}"
                        )
                        nc.scalar.activation(
                            qk_sb, qk_ps, Identity, bias=bqk_sb, scale=1.0
                        )
                        if not (SKIP_ENERGY and p > 0):
                            for ns in range(NSUB):
                                qdt = dt_of(QK_DT)
                                t_ps = pt.tile(
                                    [P, P], qdt, tag="tp", name=f"tps_{b}_{p}_{ns}"
                                )
                                nc.tensor.transpose(
                                    t_ps,
                                    qk_sb[:, ns * P : (ns + 1) * P],
                                    ident if qdt == f32 else ident_r,
                                )
                                qkt_sb = qktp.tile(
                                    [P, P],
                                    dt_of(EN_DT),
                                    tag="qkt",
                                    name=f"qkt_{b}_{p}_{ns}",
                                )
                                (
                                    nc.gpsimd if QKT_ENG == "pool" else nc.vector
                                ).tensor_copy(qkt_sb, t_ps)
                                nc.tensor.matmul(
                                    energy,
                                    qkt_sb[:, 0:C8],
                                    qkt_sb[:, C8:P],
                                    start=(p == 0 and ns == 0),
                                    stop=(p == last_p and ns == NSUB - 1),
                                )

                # ---- Phase B: softmax, W2 = Wo @ (attn/rowsum), out = W2 @ v
                negmax = smallp.tile([C8, 1], f32, tag="negmax", name=f"negmax_{b}")
                nc.vector.reduce_max(
                    negmax, energy, axis=mybir.AxisListType.X, negate=True
                )
                attn = smallp.tile([C8, C8], f32, tag="attn", name=f"attn_{b}")
                rowsum = smallp.tile([C8, 1], f32, tag="rowsum", name=f"rowsum_{b}")
                nc.scalar.activation(
                    attn, energy, Exp, bias=negmax, scale=1.0, accum_out=rowsum
                )
                recip = smallp.tile([C8, 1], f32, tag="recip", name=f"recip_{b}")
                nc.vector.reciprocal(recip, rowsum)
                # normalize attn rows (per-partition scale), typing the
                # result for the W2 matmul dtype
                zo_dt = dt_of(ZO_DT)
                if dsz(zo_dt) == 2:
                    attn_mm = smallp.tile(
                        [C8, C8], zo_dt, tag="attn2", name=f"attn2_{b}"
                    )
                    nc.vector.tensor_scalar_mul(attn_mm, attn, recip)
                    w2_rhs = wo_sb
                else:
                    nc.vector.tensor_scalar_mul(attn, attn, recip)
                    attn_mm = attn
                    w2_rhs = wo_sb.bitcast(f32)

                # W2T[d, o] = sum_c attn[c, d] WoT[c, o]  (one matmul)
                w2_ps = pt.tile([C8, C], f32, tag="tp", name=f"w2ps_{b}")
                nc.tensor.matmul(w2_ps, attn_mm, w2_rhs, start=True, stop=True)
                w2_full = zp.tile([KV, C], dt_of(ZO_DT), tag="z", name=f"w2_{b}")
                w2_sb = w2_full[0:C8, :]
                nc.vector.tensor_copy(w2_sb, w2_ps)
                if BFOLD:
                    nc.vector.tensor_copy(w2_full[C8 : C8 + 1, :], bo_row)

                def out_copy(dst, src, mo, use_act):
                    if BFOLD:
                        if use_act:
                            nc.scalar.activation(dst, src, Identity)
                        else:
                            nc.vector.tensor_copy(dst, src)
                    elif use_act:
                        nc.scalar.activation(
                            dst, src, Identity,
                            bias=bo_sb[:, mo : mo + 1], scale=1.0,
                        )
                    else:
                        nc.vector.tensor_scalar_add(
                            dst, src, bo_sb[:, mo : mo + 1]
                        )

                if SKIP_PHASEB:
                    assert dsz(dt_of(V_DT)) == dsz(y_dt), (
                        "SKIP_PHASEB timing aid needs matching v/y widths"
                    )
                    for p in range(NPANELS):
                        nsl = slice(p * NP, (p + 1) * NP)
                        nc.sync.dma_start(
                            yb[:C8, 0, nsl], v_sb[:, nsl].bitcast(y_dt)
                        )
                    continue
                if OUT_STAGE == "mo":
                    # stage one mo row-block [128, 4096] at a time: 4 DMAs
                    # per batch with 8KB-contiguous rows, pipelined over mo
                    for mo in range(CCH):
                        o_mo = op.tile([P, N], y_dt, tag="o", name=f"omo_{b}_{mo}")
                        for p in range(NPANELS):
                            nsl = slice(p * NP, (p + 1) * NP)
                            o_ps = pzo.tile(
                                [P, NP], f32, tag="zo", name=f"omps_{b}_{p}_{mo}"
                            )
                            nc.tensor.matmul(
                                o_ps,
                                w2_full[:, mo * P : (mo + 1) * P],
                                v_full[:, nsl],
                                start=True,
                                stop=True,
                            )
                            out_copy(
                                o_mo[:, nsl], o_ps, mo,
                                OUT_ENG in ("alt", "mix3") and p % 2 == 1,
                            )
                        nc.sync.dma_start(yb[:, mo, :], o_mo)
                elif OUT_STAGE == "batch":
                    # stage the whole batch's output in SBUF: one DMA per
                    # batch with 8KB-contiguous rows (vs 1KB for mtile)
                    ob_sb = op.tile([P, CCH, N], y_dt, tag="o", name=f"ob_{b}")
                    for mo in range(CCH):
                        for p in range(NPANELS):
                            nsl = slice(p * NP, (p + 1) * NP)
                            o_ps = pzo.tile(
                                [P, NP], f32, tag="zo", name=f"obps_{b}_{p}_{mo}"
                            )
                            nc.tensor.matmul(
                                o_ps,
                                w2_full[:, mo * P : (mo + 1) * P],
                                v_full[:, nsl],
                                start=True,
                                stop=True,
                            )
                            out_copy(
                                ob_sb[:, mo, nsl], o_ps, mo,
                                OUT_ENG in ("alt", "mix3") and p % 2 == 1,
                            )
                    nc.sync.dma_start(yb, ob_sb)
                elif OUT_STAGE == "panel":
                    for p in range(NPANELS):
                        nsl = slice(p * NP, (p + 1) * NP)
                        o_sb = op.tile(
                            [P, CCH, NP], y_dt, tag="o", name=f"o_{b}_{p}"
                        )
                        for mo in range(CCH):
                            o_ps = pzo.tile(
                                [P, NP], f32, tag="zo", name=f"ops_{b}_{p}_{mo}"
                            )
                            nc.tensor.matmul(
                                o_ps,
                                w2_full[:, mo * P : (mo + 1) * P],
                                v_full[:, nsl],
                                start=True,
                                stop=True,
                            )
                            out_copy(
                                o_sb[:, mo, :], o_ps, mo,
                                OUT_ENG == "act"
                                or (OUT_ENG == "alt" and mo % 2 == 1),
                            )
                        nc.sync.dma_start(yb[:, :, nsl], o_sb)
                else:
                    for mo in range(CCH):
                        for p in range(NPANELS):
                            nsl = slice(p * NP, (p + 1) * NP)
                            o_ps = pzo.tile(
                                [P, NP], f32, tag="zo", name=f"ops_{b}_{p}_{mo}"
                            )
                            nc.tensor.matmul(
                                o_ps,
                                w2_full[:, mo * P : (mo + 1) * P],
                                v_full[:, nsl],
                                start=True,
                                stop=True,
                            )
                            o_sb = op.tile(
                                [P, NP], y_dt, tag="o", name=f"o_{b}_{p}_{mo}"
                            )
                            out_copy(
                                o_sb, o_ps, mo,
                                OUT_ENG == "act"
                                or (OUT_ENG in ("alt", "mix3") and p % 2 == 1),
                            )
                            nc.sync.dma_start(yb[:, mo, nsl], o_sb)

    nc.compile()
    return nc


def _get_program():
    key = (QK_DT, V_DT, EN_DT, ZO_DT, Y_DT, SCHEME, REPS)
    if key not in _CACHE:
        _CACHE[key] = _build_program()
    return _CACHE[key]


def _np_dt(kind):
    if kind == "f16":
        return np.float16
    if kind == "bf16":
        import ml_dtypes

        return ml_dtypes.bfloat16
    return np.float32


def _host_inputs(x, Wq, bq, Wk, bk, Wv, bv, Wo, bo):
    """Build the per-core input maps (host-side shard + weight transposes)."""
    x_np = (
        _np_dt(QK_DT)
        if (_np_dt(QK_DT) != np.float32 or _np_dt(V_DT) != np.float32)
        else np.float32
    )
    x = np.ascontiguousarray(np.asarray(x, dtype=x_np).reshape(B, C, N))
    w_qkt = np.ascontiguousarray(
        np.concatenate([Wq, Wk], axis=0).T.astype(_np_dt(QK_DT))
    )  # [C, 128]
    w_vt = np.ascontiguousarray(Wv.T.astype(_np_dt(V_DT)))  # [C, 64]
    w_ot = np.ascontiguousarray(Wo.T.astype(_np_dt(ZO_DT)))  # [64, C]
    b_qk = np.ascontiguousarray(
        np.concatenate([bq, bk], axis=0).astype(np.float32)
    )  # [128]
    b_v = np.ascontiguousarray(bv.astype(np.float32))
    b_o = np.ascontiguousarray(bo.astype(np.float32))

    in_maps = []
    for i in range(NCORES):
        in_maps.append(
            {
                "x": np.ascontiguousarray(x[i * BPC : (i + 1) * BPC]),
                "w_qkt": w_qkt,
                "w_vt": w_vt,
                "w_ot": w_ot,
                "b_qk": b_qk,
                "b_v": b_v,
                "b_o": b_o,
            }
        )
    return in_maps


def kernel(**inputs):
    global LAST_RESULTS
    from concourse.bass_utils import run_bass_kernel_spmd

    nc = _get_program()
    in_maps = _host_inputs(**inputs)
    res = run_bass_kernel_spmd(nc, in_maps, core_ids=list(range(NCORES)))
    LAST_RESULTS = res
    out = np.concatenate(
        [np.asarray(r["y"]).astype(np.float32) for r in res.results], axis=0
    )
    return out.reshape(B, C, H, W)



# revision 50
# speedup vs baseline: 2.8536x; 2.8536x over previous
"""Trainium2 Bass kernel for ChannelAttention (B=16, C=512, H=W=64).

Math (per batch b):
    xf = x[b] reshaped [C, N], N = H*W = 4096
    q = Wq @ xf + bq            [64, N]
    k = Wk @ xf + bk            [64, N]
    v = Wv @ xf + bv            [64, N]
    energy = q @ k.T            [64, 64]   (contraction over N)
    attn = softmax(energy, -1)
    z = attn @ v                [64, N]
    out = Wo @ z + bo           [C, N]

Sharding: data-parallel over batch, 2 batches per core on 8 cores, no
collectives.  Each core receives its x shard plus the (host-pre-transposed)
weights and returns its out shard.

On-chip dataflow per batch (8 n-panels of 512):
  - qT|kT projected DIRECTLY in transposed [n, q|k] layout: per 128-wide
    n-subtile, a K=1 ones-row matmul seeds the PSUM with the [bq|bk] bias
    row, then 4 accumulating matmuls with the xf c-chunk as the stationary
    operand (lhsT) and [WqT|WkT] as the moving operand.  The PSUM->SBUF
    f32->f16 copies are then PURE copies, rotated across DVE/ACT/Pool by
    the QKT_PAT pattern.  The energy [64, 64] accumulates over all 32
    n-subtiles as qT.T @ kT in full fp32.
  - v projected in native [65, n] layout with an AUGMENTED weight matrix:
    column 64 of WvT is zeros and bias[64] = 1.0, so the ACT bias-copy
    produces v rows 0..63 = v and row 64 = ones "for free".  The ones row
    feeds the K=65 out matmul that folds the output bias (BFOLD).
  - softmax: DVE row-max (negated), ACT exp with bias=-max and accum_out
    row-sum, DVE reciprocal + row scale of attn in place.
  - out = Wo @ (attn @ v) + bo is reassociated as (Wo @ attn) @ v:
    W2T = attn.T-free matmul (lhsT=attn native, rhs=WoT, one instruction),
    w2_full row 64 = bo (copy), then out m-tiles = W2-slice.T @ v panels
    (K=65 folds the bias), with the PSUM->SBUF f32->f16 copies rotated
    across DVE/ACT/Pool by OUT_PAT and staged per-mo [128, 4096] before a
    single 1MB DMA per row block.
  - DMA issue engines are split so the in-order sequencers never head-of-
    line block loads behind stores: x loads issue from Pool (SWDGE),
    y stores from SP (HWDGE).

Matmul dtype notes: fp16 runs 1 cycle/row at any free dim with an 11-bit
mantissa (~tf32); measured rel L2 error ~4e-3 vs the fp32 reference,
well under the 2e-2 gate.  fp16 inputs/outputs also halve DMA traffic:
the per-core floor is ~8.4 MB in + 8.4 MB out ~= 47 us at 360 GB/s.
"""

import os

import numpy as np

# Problem shape (hardcoded; kernel.py must be self-contained).
B, C, H, W = 16, 512, 64, 64
N = H * W  # 4096
C8 = 64
KV = C8 + 1  # v rows + ones row for the out-bias fold
P = 128
NCORES = 8
BPC = B // NCORES  # batches per core
CCH = C // P  # 4 c-chunks of 128
NP = 512  # n-panel width
NPANELS = N // NP  # 8
NSUB = NP // P  # 4 transpose subtiles per panel

# Matmul dtype knobs: "f32" = exact 4 cyc/row; "f16"/"bf16" = 1 cyc/row.
QK_DT = os.environ.get("CHATT_QK_DT", "f16")
V_DT = os.environ.get("CHATT_V_DT", "f16")
EN_DT = os.environ.get("CHATT_EN_DT", "f16")
ZO_DT = os.environ.get("CHATT_ZO_DT", "f16")
# Output DRAM dtype: f16 halves, i8 quarters the store traffic.  i8 uses
# per-row dynamic scales computed on device from w2 (range = |mean| +
# QMARGIN * sigma, both exact given w2/bv/Wv-row-norms) and dequantizes
# on the host; adds ~1% quantization rel-L2 (gate is 2e-2).
Y_DT = os.environ.get("CHATT_Y_DT", "f16")
YI8 = Y_DT == "i8"
QMARGIN = float(os.environ.get("CHATT_QMARGIN", "4.5"))
# int8 rounding compensation: bias added before float->int8 conversion
# (0.5 if the hardware truncates, 0.0 if it rounds-to-nearest).
QROUND = float(os.environ.get("CHATT_QROUND", "0.0"))
# Timing aid: repeat the whole body REPS times inside a hardware loop.
REPS = int(os.environ.get("CHATT_REPS", "1"))
# Bisection aids (timing experiments only; outputs become wrong):
SKIP_ENERGY = os.environ.get("CHATT_SKIP_ENERGY", "0") == "1"
SKIP_PHASEB = os.environ.get("CHATT_SKIP_PHASEB", "0") == "1"
DMA_ONLY = os.environ.get("CHATT_DMA_ONLY", "0") == "1"
# Engine rotation patterns for PSUM->SBUF copies: v=DVE, s=ACT.
# (Pool/GPSIMD cannot access PSUM, so it only issues DMAs + memsets.)
OUT_PAT = os.environ.get("CHATT_OUT_PAT", "vs")
QKT_PAT = os.environ.get("CHATT_QKT_PAT", "vs")
V_ENG = os.environ.get("CHATT_V_ENG", "s")
# DMA issue engine rotation patterns: p=SP(HWDGE), g=Pool(SWDGE), s=ACT,
# v=DVE.  Multi-char patterns rotate queues per dma_start.
LD_ENG = os.environ.get("CHATT_LD_ENG", "g").replace("sp", "p")
ST_ENG = os.environ.get("CHATT_ST_ENG", "p").replace("sp", "p")
# Out staging: "mo" = [128, 4096] per row block (4 x 1MB DMAs per batch);
# "mtile" = [128, 512] per (mo, panel) (32 smaller DMAs).
OUT_STAGE = os.environ.get("CHATT_OUT_STAGE", "mo")
# Input DMA granularity: panels per dma_start (4 -> 2MB).
XF_PANELS = int(os.environ.get("CHATT_XF_PANELS", "4"))
# Fold the out bias via the K=65 v ones row (else bias in the out copies).
BFOLD = os.environ.get("CHATT_BFOLD", "1") == "1"
# Fold the qk bias via a K=1 ones-row matmul (else DVE tensor_tensor add).
# Default OFF: the 32 extra tiny matmuls cost ~14us/iter on HW (fixed
# per-matmul overhead the free-dim cost model does not capture).
QKBFOLD = os.environ.get("CHATT_QKBFOLD", "0") == "1"
# qk-bias handling: "dev" = unbiased qkT tiles (pure copies, rotatable
# across DVE+ACT) + on-device rank-1 energy corrections from row sums
# accumulated by an extra ones-column matmul per subtile (the 64 small
# row-sum matmuls cost ~14us/iter on HW - not worth it); "host" = same
# but the [64,64] correction comes precomputed per batch from the host
# (~2us faster than "off" but offloads an O(BCN) reduction to the host);
# "off" = per-subtile DVE tensor_tensor bias adds.  Default "off": all
# compute stays on device.
EBIAS = os.environ.get("CHATT_EBIAS", "off")
# PSUM bank split pp/pt/pe/pzo (8 total incl. the EBIAS=dev sqk bank).
PSUM_CFG = os.environ.get(
    "CHATT_PSUM", "q8" if EBIAS == "dev" else "e8"
)
# Split the first x load of batch 0 into small groups so the PE starts
# ~1.5us into the iteration instead of waiting for a full 2MB DMA.
FILL_SPLIT = os.environ.get("CHATT_FILL_SPLIT", "1") == "1"
# Split the last mo-block store into halves to shorten the drain tail.
DRAIN_SPLIT = os.environ.get("CHATT_DRAIN_SPLIT", "1") == "1"
# Staggered semaphore reset on the REPS loop: per-stage barriers instead
# of a full all-engine barrier per iteration (cross-iteration overlap).
STAG = os.environ.get("CHATT_STAG", "1") == "1"
# Manual stage boundaries at the phase junctions [A0][B0][A1][B1] so the
# next iteration's A0 loads/projections overlap this iteration's B1.
STAGM = os.environ.get("CHATT_STAGM", "0") == "1"

_CACHE = {}
LAST_RESULTS = None


def _build_program():
    import concourse.bass as bass  # noqa: F401
    import concourse.mybir as mybir
    import concourse.tile as tile
    from concourse import bacc
    from contextlib import ExitStack, nullcontext

    f32 = mybir.dt.float32
    bf16 = mybir.dt.bfloat16
    f16 = mybir.dt.float16
    y_dt = {"bf16": bf16, "f16": f16, "i8": mybir.dt.int8}.get(Y_DT, f32)
    if YI8:
        assert BFOLD and not SKIP_PHASEB, "i8 y needs BFOLD, no SKIP_PHASEB"

    def dt_of(kind):
        return {"bf16": bf16, "f16": f16}.get(kind, f32)

    dsz = mybir.dt.size
    assert dt_of(QK_DT) == dt_of(V_DT), "x feeds both projections"
    xf_dt = dt_of(QK_DT)

    nc = bacc.Bacc("TRN2", target_bir_lowering=False)

    x_h = nc.dram_tensor("x", [BPC, C, N], xf_dt, kind="ExternalInput")
    wqk_h = nc.dram_tensor("w_qkt", [C, P], dt_of(QK_DT), kind="ExternalInput")
    wv_h = nc.dram_tensor("w_vt", [C, KV], dt_of(V_DT), kind="ExternalInput")
    wo_h = nc.dram_tensor("w_ot", [C8, C], dt_of(ZO_DT), kind="ExternalInput")
    bqk_h = nc.dram_tensor("b_qk", [P], f32, kind="ExternalInput")
    bv_h = nc.dram_tensor("b_v", [KV], f32, kind="ExternalInput")
    bo_h = nc.dram_tensor("b_o", [C], f32, kind="ExternalInput")
    y_h = nc.dram_tensor("y", [BPC, C, N], y_dt, kind="ExternalOutput")
    if YI8:
        # [bv|1, rownorms2(Wv)|0] for the on-device mean/var of y rows
        bvv_h = nc.dram_tensor("bvv", [KV, 2], dt_of(ZO_DT), kind="ExternalInput")
        ys_h = nc.dram_tensor("y_s", [BPC, C], f32, kind="ExternalOutput")
    if EBIAS == "host":
        # per-batch rank-1 energy bias corrections, precomputed on host
        ecorr_h = nc.dram_tensor(
            "ecorr", [BPC, C8, C8], f32, kind="ExternalInput"
        )

    x_ap = x_h.ap()
    y_ap = y_h.ap()

    with tile.TileContext(nc) as tc, ExitStack() as ctx:
        def _n(name, default):
            return int(os.environ.get(f"CHATT_BUFS_{name}", str(default)))

        consts = ctx.enter_context(tc.tile_pool(name="consts", bufs=1))
        xp = ctx.enter_context(
            tc.tile_pool(name="xp", bufs=_n("XP", max(3, 8 // XF_PANELS)))
        )
        qktp = ctx.enter_context(tc.tile_pool(name="qktp", bufs=_n("QKTP", 6)))
        vp = ctx.enter_context(tc.tile_pool(name="vp", bufs=_n("VP", 2)))
        zp = ctx.enter_context(tc.tile_pool(name="zp", bufs=3))
        op = ctx.enter_context(
            tc.tile_pool(
                name="op",
                bufs=_n("OP", {"mo": 3, "mtile": 6}.get(OUT_STAGE, 3)),
            )
        )
        smallp = ctx.enter_context(tc.tile_pool(name="smallp", bufs=4))
        # PSUM: 8 banks total: pp (v proj) / pt (qkT) / pe (energy) / pzo (out)
        pp_n, pt_n, pe_n, pzo_n = {
            "a8": (2, 3, 1, 2),
            "b8": (2, 2, 2, 2),
            "c8": (1, 3, 2, 2),
            "d8": (2, 2, 1, 3),
            "e8": (1, 3, 1, 3),
            "q8": (1, 3, 1, 2),
            "r8": (1, 2, 1, 3),
            "t8": (1, 4, 1, 2),
            "u8": (2, 4, 1, 1),
        }[PSUM_CFG]
        pp = ctx.enter_context(tc.tile_pool(name="pp", bufs=pp_n, space="PSUM"))
        pt = ctx.enter_context(tc.tile_pool(name="pt", bufs=pt_n, space="PSUM"))
        pe = ctx.enter_context(tc.tile_pool(name="pe", bufs=pe_n, space="PSUM"))
        pzo = ctx.enter_context(tc.tile_pool(name="pzo", bufs=pzo_n, space="PSUM"))
        if EBIAS == "dev":
            # dedicated bank for the [sq0|sk0] accumulator: PSUM start/stop
            # is bank-granular, so it cannot share the energy bank
            psq = ctx.enter_context(
                tc.tile_pool(name="psq", bufs=1, space="PSUM")
            )

        # One-time constants.
        wqk_sb = consts.tile([P, CCH, P], dt_of(QK_DT))
        nc.sync.dma_start(wqk_sb, wqk_h.ap().rearrange("(co ci) m -> ci co m", ci=P))
        wv_sb = consts.tile([P, CCH, KV], dt_of(V_DT))
        nc.sync.dma_start(wv_sb, wv_h.ap().rearrange("(co ci) m -> ci co m", ci=P))
        wo_sb = consts.tile([C8, C], dt_of(ZO_DT))
        nc.sync.dma_start(wo_sb, wo_h.ap())
        bv_sb = consts.tile([KV, 1], f32)
        nc.sync.dma_start(bv_sb, bv_h.ap()[:, None])
        bo_sb = consts.tile([P, CCH], f32)
        nc.sync.dma_start(bo_sb, bo_h.ap().rearrange("(mo mi) -> mi mo", mi=P))
        # bo as a [1, C] row in the out-matmul dtype, for the K=65 fold
        bo_row32 = consts.tile([1, C], f32)
        nc.sync.dma_start(bo_row32, bo_h.ap()[None, :])
        bo_row = consts.tile([1, C], dt_of(ZO_DT))
        nc.vector.tensor_copy(bo_row, bo_row32)
        if YI8:
            bvv_sb = consts.tile([KV, 2], dt_of(ZO_DT))
            nc.sync.dma_start(bvv_sb, bvv_h.ap())
        if QKBFOLD or EBIAS != "off":
            # [bq|bk] as a [1, 128] row in the qk matmul dtype
            ones_row = consts.tile([1, P], dt_of(QK_DT))
            nc.gpsimd.memset(ones_row, 1.0)
            bqk_row32 = consts.tile([1, P], f32)
            nc.sync.dma_start(bqk_row32, bqk_h.ap()[None, :])
            bqk_row = consts.tile([1, P], dt_of(QK_DT))
            nc.vector.tensor_copy(bqk_row, bqk_row32)
        if EBIAS == "dev":
            # ones column (sqk row-sum matmul stationary) and N*bk row
            ones_col = consts.tile([P, 1], dt_of(EN_DT))
            nc.gpsimd.memset(ones_col, 1.0)
            bkn_row = consts.tile([1, C8], dt_of(EN_DT))
            nc.vector.tensor_scalar_mul(
                bkn_row, bqk_row32[:, C8:P], float(N)
            )
        if EBIAS == "off" and not QKBFOLD:
            # b_qk broadcast to all partitions: [128, 128] with the bias
            # along the free dim (for the transposed-layout bias add)
            bqk_bc = consts.tile([P, P], f32)
            nc.sync.dma_start(
                bqk_bc,
                bass.AP(tensor=bqk_h, offset=0, ap=[[0, P], [1, P]]),
            )

        Identity = mybir.ActivationFunctionType.Identity
        Exp = mybir.ActivationFunctionType.Exp

        dma_eng = {
            "p": nc.sync,
            "g": nc.gpsimd,
            "s": nc.scalar,
            "v": nc.vector,
        }
        ld_count = [0]
        st_count = [0]

        class _EngRotor:
            def __init__(self, pat, counter):
                self.pat = pat
                self.counter = counter

            def dma_start(self, out, in_):
                eng = dma_eng[self.pat[self.counter[0] % len(self.pat)]]
                self.counter[0] += 1
                return eng.dma_start(out, in_)

        ld_eng = _EngRotor(LD_ENG, ld_count)
        st_eng = _EngRotor(ST_ENG, st_count)

        def copy_to(dst, src, eng, bias=None, scale=None):
            # PSUM->SBUF copy (+ optional per-partition bias or quantize
            # scale) on a chosen engine: v=DVE, s=ACT
            if scale is not None:
                if eng == "s":
                    nc.scalar.activation(
                        dst, src, Identity, bias=QROUND, scale=scale
                    )
                elif QROUND != 0.0:
                    nc.vector.tensor_scalar(
                        dst, src, scale, QROUND,
                        mybir.AluOpType.mult, mybir.AluOpType.add,
                    )
                else:
                    nc.vector.tensor_scalar_mul(dst, src, scale)
            elif bias is None:
                if eng == "s":
                    nc.scalar.activation(dst, src, Identity)
                elif eng == "g":
                    nc.gpsimd.tensor_copy(dst, src)
                else:
                    nc.vector.tensor_copy(dst, src)
            else:
                if eng == "s":
                    nc.scalar.activation(dst, src, Identity, bias=bias, scale=1.0)
                elif eng == "g":
                    nc.gpsimd.tensor_scalar_add(dst, src, bias)
                else:
                    nc.vector.tensor_scalar_add(dst, src, bias)

        def proj_panel(b, p, xf, energy, v_full, nqkt, sqk_ps=None):
            """v projection + qkT subtiles + energy accumulation for one
            512-wide n panel.  xf is the [128, CCH, 512] slice of the
            loaded x group."""
            nsl = slice(p * NP, (p + 1) * NP)
            v_ps = pp.tile([KV, NP], f32, tag="proj", name=f"vps_{b}_{p}")
            for co in range(CCH):
                nc.tensor.matmul(
                    v_ps,
                    wv_sb[:, co, :],
                    xf[:, co, :],
                    start=(co == 0),
                    stop=(co == CCH - 1),
                )
            copy_to(v_full[:, nsl], v_ps, V_ENG, bias=bv_sb)

            last_p = 0 if SKIP_ENERGY else NPANELS - 1
            if SKIP_ENERGY and p > 0:
                return
            for ns in range(NSUB):
                qt_ps = pt.tile([P, P], f32, tag="tp", name=f"qtps_{b}_{p}_{ns}")
                if QKBFOLD:
                    nc.tensor.matmul(
                        qt_ps, ones_row, bqk_row, start=True, stop=False
                    )
                for co in range(CCH):
                    nc.tensor.matmul(
                        qt_ps,
                        xf[:, co, ns * P : (ns + 1) * P],
                        wqk_sb[:, co, :],
                        start=(co == 0 and not QKBFOLD),
                        stop=(co == CCH - 1),
                    )
                qkt_sb = qktp.tile(
                    [P, P], dt_of(EN_DT), tag="qkt", name=f"qkt_{b}_{p}_{ns}"
                )
                if QKBFOLD or EBIAS != "off":
                    copy_to(qkt_sb, qt_ps, QKT_PAT[(nqkt + ns) % len(QKT_PAT)])
                else:
                    nc.vector.tensor_tensor(
                        qkt_sb, qt_ps, bqk_bc, mybir.AluOpType.add
                    )
                first = p == 0 and ns == 0
                last = p == last_p and ns == NSUB - 1
                nc.tensor.matmul(
                    energy,
                    qkt_sb[:, 0:C8],
                    qkt_sb[:, C8:P],
                    start=first,
                    # with EBIAS=dev the accumulation ends at the rank-1
                    # bias-correction matmuls emitted after the last panel
                    stop=last and EBIAS != "dev",
                )
                if EBIAS == "dev":
                    nc.tensor.matmul(
                        sqk_ps, ones_col, qkt_sb, start=first, stop=last
                    )

        rep_cm = (
            tc.For_i(0, REPS, 1, staggered_reset=STAG)
            if REPS > 1
            else nullcontext()
        )
        with rep_cm:
            if DMA_ONLY:
                # Pure-DMA roofline probe: full x loads + full y stores of
                # a constant tile; no compute dependencies.
                dummy = consts.tile([P, N], y_dt)
                nc.gpsimd.memset(dummy, 0.25)
                for b in range(BPC):
                    xb = x_ap[b].rearrange("(co ci) n -> ci co n", ci=P)
                    yb = y_ap[b].rearrange("(mo mi) n -> mi mo n", mi=P)
                    for p in range(0, NPANELS, XF_PANELS):
                        gw = XF_PANELS * NP
                        xf_g = xp.tile(
                            [P, CCH, gw], xf_dt, tag="xf", name=f"xf_{b}_{p}"
                        )
                        ld_eng.dma_start(xf_g, xb[:, :, p * NP : p * NP + gw])
                    for mo in range(CCH):
                        st_eng.dma_start(yb[:, mo, :], dummy)
            for b in range(BPC if not DMA_ONLY else 0):
                xb = x_ap[b].rearrange("(co ci) n -> ci co n", ci=P)
                yb = y_ap[b].rearrange("(mo mi) n -> mi mo n", mi=P)

                energy = pe.tile(
                    [C8, C8], f32, tag="energy", name=f"energy_{b}"
                )
                sqk_ps = (
                    psq.tile([1, P], f32, tag="sqk", name=f"sqkps_{b}")
                    if EBIAS == "dev"
                    else None
                )
                if EBIAS == "host":
                    ecorr_sb = smallp.tile(
                        [C8, C8], f32, tag="ecorr", name=f"ecorr_{b}"
                    )
                    ld_eng.dma_start(ecorr_sb, ecorr_h.ap()[b])
                v_full = vp.tile([KV, N], dt_of(ZO_DT), tag="v", name=f"v_{b}")

                # ---- Phase A: projections + energy accumulation ----
                # x load group sizes (panels per dma_start): batch 0 ramps
                # up from small groups so the PE starts early in the
                # iteration (the For_i loop re-barriers every iteration).
                if b == 0 and FILL_SPLIT:
                    sizes, left = [1, 1, 2], NPANELS - 4
                else:
                    sizes, left = [], NPANELS
                while left > 0:
                    g = min(XF_PANELS, left)
                    sizes.append(g)
                    left -= g
                panel_of = []
                p0 = 0
                for g in sizes:
                    panel_of.append((p0, g))
                    p0 += g

                nqkt = 0
                for p0, g in panel_of:
                    gw = g * NP
                    xf_g = xp.tile(
                        [P, CCH, gw], xf_dt, tag="xf", name=f"xf_{b}_{p0}"
                    )
                    ld_eng.dma_start(xf_g, xb[:, :, p0 * NP : p0 * NP + gw])
                    for pi in range(g):
                        p = p0 + pi
                        proj_panel(
                            b, p, xf_g[:, :, pi * NP : (pi + 1) * NP],
                            energy, v_full, nqkt, sqk_ps=sqk_ps,
                        )
                        if not (SKIP_ENERGY and p > 0):
                            nqkt += NSUB

                if EBIAS == "dev":
                    # rank-1 bias corrections close the energy accumulation:
                    # energy += bq (sk0 + N bk)^T + sq0 bk^T
                    en_dt = dt_of(EN_DT)
                    sqk_sb = smallp.tile([1, P], en_dt, tag="sqk", name=f"sqk_{b}")
                    nc.scalar.activation(sqk_sb, sqk_ps, Identity)
                    ck = smallp.tile([1, C8], en_dt, tag="sqk", name=f"ck_{b}")
                    nc.vector.tensor_tensor(
                        ck, sqk_sb[:, C8:P], bkn_row, mybir.AluOpType.add
                    )
                    nc.tensor.matmul(
                        energy, bqk_row[:, 0:C8], ck, start=False, stop=False
                    )
                    nc.tensor.matmul(
                        energy, sqk_sb[:, 0:C8], bqk_row[:, C8:P],
                        start=False, stop=True,
                    )

                if EBIAS == "host":
                    esum = smallp.tile(
                        [C8, C8], f32, tag="esum", name=f"esum_{b}"
                    )
                    nc.vector.tensor_tensor(
                        esum, energy, ecorr_sb, mybir.AluOpType.add
                    )
                    eread = esum
                else:
                    eread = energy

                if STAGM and STAG and REPS > 1:
                    tc.stage_boundary()  # end of phase A for this batch

                # ---- Phase B: softmax, W2 = Wo @ (attn/rowsum), out = W2 @ v
                negmax = smallp.tile([C8, 1], f32, tag="negmax", name=f"negmax_{b}")
                nc.vector.reduce_max(
                    negmax, eread, axis=mybir.AxisListType.X, negate=True
                )
                attn = smallp.tile([C8, C8], f32, tag="attn", name=f"attn_{b}")
                rowsum = smallp.tile([C8, 1], f32, tag="rowsum", name=f"rowsum_{b}")
                nc.scalar.activation(
                    attn, eread, Exp, bias=negmax, scale=1.0, accum_out=rowsum
                )
                recip = smallp.tile([C8, 1], f32, tag="recip", name=f"recip_{b}")
                nc.vector.reciprocal(recip, rowsum)
                zo_dt = dt_of(ZO_DT)
                attn_mm = smallp.tile([C8, C8], zo_dt, tag="attn2", name=f"attn2_{b}")
                nc.vector.tensor_scalar_mul(attn_mm, attn, recip)

                # W2T[d, o] = sum_c attn[c, d] WoT[c, o]  (one matmul)
                w2_ps = pt.tile([C8, C], f32, tag="tp", name=f"w2ps_{b}")
                nc.tensor.matmul(w2_ps, attn_mm, wo_sb, start=True, stop=True)
                w2_full = zp.tile([KV, C], zo_dt, tag="z", name=f"w2_{b}")
                nc.vector.tensor_copy(w2_full[0:C8, :], w2_ps)
                if BFOLD:
                    nc.scalar.activation(
                        w2_full[C8 : C8 + 1, :], bo_row, Identity
                    )
                kv_mm = KV if BFOLD else C8

                recips_t = None
                if YI8:
                    # Per-row int8 range: |mean| + QMARGIN * sigma, where
                    # mean_c = sum_d w2[d,c]*E[v_d] (E[v]=[bv|1]) and
                    # var_c = sum_d w2[d,c]^2 * Var(v_d) (Var=[|Wv_d|^2|0])
                    w2sq = zp.tile([KV, C], zo_dt, tag="z2", name=f"w2sq_{b}")
                    nc.vector.tensor_tensor(
                        w2sq, w2_full, w2_full, mybir.AluOpType.mult
                    )
                    scales_t = smallp.tile(
                        [P, CCH], f32, tag="scl", name=f"scl_{b}"
                    )
                    recips_t = smallp.tile(
                        [P, CCH], f32, tag="rcp", name=f"rcp_{b}"
                    )
                    Abs = mybir.ActivationFunctionType.Abs
                    Sqrt = mybir.ActivationFunctionType.Sqrt
                    for mo in range(CCH):
                        msl = slice(mo * P, (mo + 1) * P)
                        # pzo (phase-B) pool: keeps the pt ring free for the
                        # next batch's qkT tiles
                        mv_ps = pzo.tile([P, 2], f32, tag="zo", name=f"mv_{b}_{mo}")
                        nc.tensor.matmul(
                            mv_ps[:, 0:1], w2_full[0:KV, msl], bvv_sb[:, 0:1],
                            start=True, stop=True,
                        )
                        nc.tensor.matmul(
                            mv_ps[:, 1:2], w2sq[0:KV, msl], bvv_sb[:, 1:2],
                            start=True, stop=True,
                        )
                        sd = smallp.tile([P, 1], f32, tag="qtmp", name=f"sd_{b}_{mo}")
                        nc.scalar.activation(
                            sd, mv_ps[:, 1:2], Sqrt,
                            scale=(QMARGIN / 127.0) ** 2,
                        )
                        nc.scalar.activation(
                            scales_t[:, mo : mo + 1], mv_ps[:, 0:1], Abs,
                            scale=1.0 / 127.0,
                        )
                        nc.vector.tensor_tensor(
                            scales_t[:, mo : mo + 1], scales_t[:, mo : mo + 1],
                            sd, mybir.AluOpType.add,
                        )
                        nc.vector.reciprocal(
                            recips_t[:, mo : mo + 1], scales_t[:, mo : mo + 1]
                        )
                    st_eng.dma_start(
                        ys_h.ap()[b].rearrange("(mo mi) -> mi mo", mi=P),
                        scales_t,
                    )

                if SKIP_PHASEB:
                    for p in range(NPANELS):
                        nsl = slice(p * NP, (p + 1) * NP)
                        nc.sync.dma_start(
                            yb[:C8, 0, nsl], v_full[0:C8, nsl].bitcast(y_dt)
                        )
                    continue
                nout = 0
                if OUT_STAGE == "mo":
                    # stage one mo row-block [128, 4096] at a time: 4 DMAs
                    # per batch with 8KB-contiguous rows, pipelined over mo
                    for mo in range(CCH):
                        o_mo = op.tile([P, N], y_dt, tag="o", name=f"omo_{b}_{mo}")
                        for p in range(NPANELS):
                            nsl = slice(p * NP, (p + 1) * NP)
                            o_ps = pzo.tile(
                                [P, NP], f32, tag="zo", name=f"omps_{b}_{p}_{mo}"
                            )
                            nc.tensor.matmul(
                                o_ps,
                                w2_full[0:kv_mm, mo * P : (mo + 1) * P],
                                v_full[0:kv_mm, nsl],
                                start=True,
                                stop=True,
                            )
                            copy_to(
                                o_mo[:, nsl], o_ps,
                                OUT_PAT[nout % len(OUT_PAT)],
                                bias=None if BFOLD else bo_sb[:, mo : mo + 1],
                                scale=None if recips_t is None
                                else recips_t[:, mo : mo + 1],
                            )
                            nout += 1
                        if DRAIN_SPLIT and b == BPC - 1 and mo == CCH - 1:
                            # split the final store so its first half can
                            # fire while the last panels are still copying
                            st_eng.dma_start(
                                yb[:, mo, 0 : N // 2], o_mo[:, 0 : N // 2]
                            )
                            st_eng.dma_start(
                                yb[:, mo, N // 2 : N], o_mo[:, N // 2 : N]
                            )
                        else:
                            st_eng.dma_start(yb[:, mo, :], o_mo)
                else:
                    for mo in range(CCH):
                        for p in range(NPANELS):
                            nsl = slice(p * NP, (p + 1) * NP)
                            o_ps = pzo.tile(
                                [P, NP], f32, tag="zo", name=f"ops_{b}_{p}_{mo}"
                            )
                            nc.tensor.matmul(
                                o_ps,
                                w2_full[0:kv_mm, mo * P : (mo + 1) * P],
                                v_full[0:kv_mm, nsl],
                                start=True,
                                stop=True,
                            )
                            o_sb = op.tile(
                                [P, NP], y_dt, tag="o", name=f"o_{b}_{p}_{mo}"
                            )
                            copy_to(
                                o_sb, o_ps,
                                OUT_PAT[nout % len(OUT_PAT)],
                                bias=None if BFOLD else bo_sb[:, mo : mo + 1],
                                scale=None if recips_t is None
                                else recips_t[:, mo : mo + 1],
                            )
                            nout += 1
                            st_eng.dma_start(yb[:, mo, nsl], o_sb)
                if STAGM and STAG and REPS > 1 and b < BPC - 1:
                    tc.stage_boundary()  # end of phase B for this batch

    nc.compile()
    return nc


def _get_program():
    key = (
        QK_DT, V_DT, EN_DT, ZO_DT, Y_DT, REPS, OUT_PAT, QKT_PAT, V_ENG,
        LD_ENG, ST_ENG, OUT_STAGE, XF_PANELS, BFOLD, QKBFOLD, PSUM_CFG,
        SKIP_ENERGY, SKIP_PHASEB, DMA_ONLY, FILL_SPLIT, DRAIN_SPLIT, STAG,
        STAGM, QMARGIN, QROUND, EBIAS,
    )
    if key not in _CACHE:
        _CACHE[key] = _build_program()
    return _CACHE[key]


def _np_dt(kind):
    if kind == "f16":
        return np.float16
    if kind == "bf16":
        import ml_dtypes

        return ml_dtypes.bfloat16
    return np.float32


def _host_inputs(x, Wq, bq, Wk, bk, Wv, bv, Wo, bo):
    """Build the per-core input maps (host-side shard + weight transposes)."""
    x = np.ascontiguousarray(np.asarray(x, dtype=_np_dt(QK_DT)).reshape(B, C, N))
    w_qkt = np.ascontiguousarray(
        np.concatenate([Wq, Wk], axis=0).T.astype(_np_dt(QK_DT))
    )  # [C, 128]
    # augmented WvT: column 64 is zeros, bias row 64 is 1.0 -> v row 64 = ones
    w_vt = np.ascontiguousarray(
        np.concatenate(
            [Wv.T, np.zeros((C, 1), np.float32)], axis=1
        ).astype(_np_dt(V_DT))
    )  # [C, 65]
    w_ot = np.ascontiguousarray(Wo.T.astype(_np_dt(ZO_DT)))  # [64, C]
    b_qk = np.ascontiguousarray(
        np.concatenate([bq, bk], axis=0).astype(np.float32)
    )  # [128]
    b_v = np.ascontiguousarray(
        np.concatenate([bv, np.ones((1,), np.float32)]).astype(np.float32)
    )  # [65]
    b_o = np.ascontiguousarray(bo.astype(np.float32))

    in_maps = []
    for i in range(NCORES):
        m = {
            "x": np.ascontiguousarray(x[i * BPC : (i + 1) * BPC]),
            "w_qkt": w_qkt,
            "w_vt": w_vt,
            "w_ot": w_ot,
            "b_qk": b_qk,
            "b_v": b_v,
            "b_o": b_o,
        }
        if YI8:
            vvar = (np.asarray(Wv, np.float64) ** 2).sum(axis=1)
            col0 = np.concatenate([np.asarray(bv, np.float64), [1.0]])
            col1 = np.concatenate([vvar, [0.0]])
            m["bvv"] = np.ascontiguousarray(
                np.stack([col0, col1], axis=1).astype(_np_dt(ZO_DT))
            )
        if EBIAS == "host":
            # energy = q0 k0^T + bq (Wk sx + N bk)^T + (Wq sx) bk^T,
            # sx = x.sum(n)
            xs = m["x"].astype(np.float32).sum(axis=2)  # [BPC, C]
            wq = np.asarray(Wq, np.float32)
            wk = np.asarray(Wk, np.float32)
            ec = (
                np.asarray(bq, np.float32)[None, :, None]
                * (xs @ wk.T + N * np.asarray(bk, np.float32))[:, None, :]
                + (xs @ wq.T)[:, :, None]
                * np.asarray(bk, np.float32)[None, None, :]
            )
            m["ecorr"] = np.ascontiguousarray(ec.astype(np.float32))
        in_maps.append(m)
    return in_maps


def kernel(**inputs):
    global LAST_RESULTS
    from concourse.bass_utils import run_bass_kernel_spmd

    nc = _get_program()
    in_maps = _host_inputs(**inputs)
    res = run_bass_kernel_spmd(nc, in_maps, core_ids=list(range(NCORES)))
    LAST_RESULTS = res
    if YI8:
        out = np.concatenate(
            [
                np.asarray(r["y"]).astype(np.float32)
                * np.asarray(r["y_s"])[:, :, None]
                for r in res.results
            ],
            axis=0,
        )
    else:
        out = np.concatenate(
            [np.asarray(r["y"]).astype(np.float32) for r in res.results], axis=0
        )
    return out.reshape(B, C, H, W)
